# revision 46
# baseline (speedup 1.0000x reference)
"""GAT layer (nn_GATLayer_44220983279640) — Trainium2 Bass/Tile kernel.

Reference math per graph (B=16, D=512, FIN=FOUT=128, H=8):
    h  = x @ W                                         [D, F]
    s1[hd,i] = h[i] . a1[hd]   s2[hd,j] = h[j] . a2[hd]
    e  = leaky_relu(s1[:,None] + s2[None,:] + ab)      [H, D, D]
    att = softmax_j(where(adj > 0, e, -9e15))
    out = mean_hd(att @ h)                             [D, F]

Sharding: data-parallel over batch, 2 graphs per core on 8 cores.

Design (measured ~90us/core vs the 103us 3-pass baseline):
  * custom DVE op GAT_LRELU_MASK (registered into concourse.dve_ops at
    build time, sha computed dynamically): out = max(w, 0.01*w) with
    w = (maskT + s2col) + s1b — the entire leaky_relu(logits+mask)
    construction in ONE 1x DVE pass per head, deleting the baseline's
    ACT Prelu pass. E^T layout [j, i] keeps s2 per-partition (scalar
    slot) and s1 as a DMA row-broadcast; s1 rows stage through DRAM and
    all 16 broadcasts are issued at setup into persistent tiles.
  * ACT does one Exp [128,2048] per head (bias = 8 - bound, per-head,
    broadcast via a tiny DRAM-roundtrip stride-0 DMA) producing fp16 E.
  * aggregation: per i-tile matmuls vs [h16 | 8.0] (FD=129); the 8.0
    ones-column folds the mean-over-heads into the rowsum; two i-tiles
    share each PSUM bank; reciprocal over both rowsum columns at once.
  * merge: 3 of 4 tiles scaled U*(1/8R) on ACT (Copy with per-partition
    AP scale), 1 on DVE tensor_scalar (this 3/1 split is a measured
    optimum — 2/2 and 4/0 are both ~25% slower), all into one fp16 tmp;
    head-accumulation is a PE identity-matmul into a per-graph PSUM
    bank (start at hd==0, stop at hd==7) — f32 accumulation for free,
    no DVE tensor_tensor chain.
  * mask DMAs issue after all other setup: input packets round-robin
    across the shared DMA engines, so deferring the late-needed masks
    lets the critical consts/x tensors complete first.
  * remaining wall time: ~7us fixed NEFF preamble, ~14us input-DMA +
    setup ramp before the pipeline saturates, then DVE (fused op, 1x,
    ~740ns/[128,512]) and ACT (Exp + merges) both ~95% busy. Custom DVE
    ops and scalar_tensor_tensor have no 2x uops on cayman, so the
    logit pass is the hard floor of this structure. Measured optima are
    sharp: merge split 2/2 or 4/0, a both-graphs-wide Exp (bias via the
    fused op's C1 slot), or DVE-side setup evictions each regress
    10-25% by disrupting the schedule; runs show occasional +15us
    outliers, so every decision here was confirmed with repeat runs.
"""

from contextlib import ExitStack

import numpy as np

import concourse.bass as bass
import concourse.bacc as bacc
import concourse.tile as tile
from concourse import mybir
from concourse.bass_utils import run_bass_kernel_spmd

B, D, FIN, FOUT, H = 16, 512, 128, 128, 8
NCORES = 8
NB = B // NCORES          # graphs per core
P = 128                   # partitions
NCH = D // P              # 4 j-chunks / i-tiles
NEG = -9.0e15

F32 = mybir.dt.float32
F16 = mybir.dt.float16
BF16 = mybir.dt.bfloat16

# packed consts layout (columns): W | W^T | aT | ab | selmat | id8
CONST_COLS = 2 * FOUT + 2 * H + 1 + H + P  # W|WT|aT|ab|id8|I128

# how many of the 4 per-head merge-scales run on ACT (rest on DVE stt)
N_ACT_MERGE = 3

_NC_CACHE = {}


def _register_fused_op():
    """Register GAT_LRELU_MASK: out = max(w, imm2*w), w = (in0 + s0) + in1.

    in0 = maskT chunk (bf16, 0 / -9e15), s0 = s2 column [P,1] f32,
    in1 = s1 row-broadcast [128, D] f32, imm2 = 0.01. TTSS shape (in1 has
    one free dim) so imm2 is available. f32 internal math, any out dtype.
    """
    import concourse.dve_ops as dve_ops
    from concourse.dve_spec import Spec, Src0, Src1, C0, C1, C2, maxx, lower, _has_src1
    from concourse.dve_uop import DveOpSpec

    name = "GAT_LRELU_MASK"
    if any(op.name == name for op in dve_ops.OPS):
        return next(op for op in dve_ops.OPS if op.name == name)

    _w = (Src0 + C0) + Src1
    spec = Spec(
        body=maxx(_w, _w * C2),
        reference=lambda in0, in1, s0, s1, imm2: (
            lambda w: np.maximum(w, w * imm2)
        )(in0.astype(np.float32) + s0 + in1),
    )
    row = max(dve_ops._SUB_OPCODE_FOR_NAME.values()) + 1
    assert row < 0x20
    dve_ops._SUB_OPCODE_FOR_NAME[name] = row
    # compute uops_sha for all vers so DveOp.compile's drift check passes
    shas = {}
    for ver in ("v3", "v4"):
        s = DveOpSpec(name=name, opcode=row, uops=lower(spec, ver=ver),
                      rd1_en=_has_src1(spec))
        shas[ver] = s.sha(ver)
    op = dve_ops.DveOp(name=name, spec=spec, subdim=False, uops_sha=shas)
    dve_ops.OPS.append(op)
    dve_ops.CUSTOM_DVE_SPECS[name] = spec
    return op


def _build_bass():
    fused_op = _register_fused_op()
    nc = bacc.Bacc("TRN2", debug=False, num_devices=NCORES)

    xT = nc.dram_tensor("xT", [NB, FIN, D], F32, kind="ExternalInput").ap()
    maskT = nc.dram_tensor("maskT", [NB, NCH, P, D], BF16, kind="ExternalInput").ap()
    consts = nc.dram_tensor("consts", [P, CONST_COLS], F32, kind="ExternalInput").ap()
    s1d = nc.dram_tensor("s1d", [NB, H, D], F32).ap()
    nbd = nc.dram_tensor("nbd", [NB, H], F32).ap()
    out = nc.dram_tensor("out", [NB, P, NCH * FOUT], F16, kind="ExternalOutput").ap()

    with tile.TileContext(nc) as tc, ExitStack() as ctx:
        _kernel_body(ctx, tc, out, xT, maskT, consts, s1d, nbd, fused_op)
    nc.compile()
    return nc


def _kernel_body(ctx, tc, out, xT, maskT, consts, s1d, nbd, fused_op):
    nc = tc.nc
    add, mult = mybir.AluOpType.add, mybir.AluOpType.mult

    const = ctx.enter_context(tc.tile_pool(name="const", bufs=1))
    xpool = ctx.enter_context(tc.tile_pool(name="xpool", bufs=NB))
    mpool = ctx.enter_context(tc.tile_pool(name="mpool", bufs=2 * NCH))
    spool = ctx.enter_context(tc.tile_pool(name="spool", bufs=NB))
    s2tpool = ctx.enter_context(tc.tile_pool(name="s2tpool", bufs=2 * NCH))
    upool = ctx.enter_context(tc.tile_pool(name="upool", bufs=10))
    epool = ctx.enter_context(tc.tile_pool(name="epool", bufs=10))
    s1bpool = ctx.enter_context(tc.tile_pool(name="s1bpool", bufs=NB * H))
    hpool = ctx.enter_context(tc.tile_pool(name="hpool", bufs=2 * NCH))
    apool = ctx.enter_context(tc.tile_pool(name="apool", bufs=2))
    rpool = ctx.enter_context(tc.tile_pool(name="rpool", bufs=12))
    tpool = ctx.enter_context(tc.tile_pool(name="tpool", bufs=6))
    # PSUM: setup scratch 2 + U bank-pairs 6 (2 i-tiles per bank, FD=129)
    pset = ctx.enter_context(tc.tile_pool(name="pset", bufs=2, space="PSUM"))
    pout = ctx.enter_context(tc.tile_pool(name="pout", bufs=4, space="PSUM"))
    pacc = ctx.enter_context(tc.tile_pool(name="pacc", bufs=2, space="PSUM"))

    # warm the ACT table set at t=0 (hoists the ~2.7us ACT_TABLE_LOAD off
    # the setup critical path; depends on nothing but a 1-elem memset)
    warm = const.tile([1, 1], F32)
    nc.vector.memset(warm, 0.0)
    nc.scalar.activation(warm[:], warm[:], mybir.ActivationFunctionType.Exp)

    # --- constants (one packed DMA; see _pack_consts for the layout) -------
    cst = const.tile([P, CONST_COLS], F32)
    nc.sync.dma_start(out=cst, in_=consts)
    W_sb = cst[:, 0:FOUT]
    WT_sb = cst[:, FOUT : 2 * FOUT]
    aT_sb = cst[:, 2 * FOUT : 2 * FOUT + 2 * H]
    ab_sb = cst[0:H, 2 * FOUT + 2 * H : 2 * FOUT + 2 * H + 1]
    SEL0 = 2 * FOUT + 2 * H + 1
    ident8 = cst[0:H, SEL0 : SEL0 + H]
    ident128_f32 = cst[:, SEL0 + H : SEL0 + H + P]



    I128 = const.tile([P, P], F16)
    nc.scalar.activation(I128[:], ident128_f32, mybir.ActivationFunctionType.Copy)

    # Wa[fin, 0:8]=W@a1^T, [fin, 8:16]=W@a2^T  (shared across graphs)
    p_wa = pset.tile([P, D], F32, tag="setup")
    nc.tensor.matmul(p_wa[:, 0 : 2 * H], WT_sb, aT_sb, start=True, stop=True)
    Wa_sb = const.tile([FIN, 2 * H], F32)
    nc.scalar.activation(Wa_sb[:], p_wa[:, 0 : 2 * H], mybir.ActivationFunctionType.Copy)

    G = []  # per-graph setup state
    for b in range(NB):
        # --- per-graph setup ----------------------------------------------
        x_sb = xpool.tile([FIN, D], F32, tag="x")
        nc.sync.dma_start(out=x_sb, in_=xT[b])

        # s1/s2 for all heads: [8, D] each
        p_s1 = pset.tile([P, D], F32, tag="setup")
        nc.tensor.matmul(p_s1[0:H, :], Wa_sb[:, 0:H], x_sb[:], start=True, stop=True)
        s1_sb = spool.tile([H, D], F32, tag="s1")
        nc.scalar.activation(s1_sb[:], p_s1[0:H, :], mybir.ActivationFunctionType.Copy)
        # stage s1 rows in DRAM, then immediately row-broadcast each head
        # back into a persistent [P, H*D] tile: all 16 broadcast DMAs are
        # issued during setup so the head loop never waits on DMA supply
        nc.sync.dma_start(out=s1d[b], in_=s1_sb[:])
        s1ball = []
        for hd in range(H):
            s1bt = s1bpool.tile([P, D], F32, tag="s1b")
            s1row = s1d[b, hd]
            nc.gpsimd.dma_start(
                out=s1bt[:],
                in_=bass.AP(
                    tensor=s1d.tensor, offset=s1row.offset,
                    ap=[[0, P], s1row.ap[-1]],
                ),
            )
            s1ball.append(s1bt)
        p_s2 = pset.tile([P, D], F32, tag="setup")
        nc.tensor.matmul(
            p_s2[0:H, :], Wa_sb[:, H : 2 * H], x_sb[:], start=True, stop=True
        )
        s2b_sb = spool.tile([H, D], F32, tag="s2")
        nc.scalar.activation(
            s2b_sb[:], p_s2[0:H, :], mybir.ActivationFunctionType.Identity,
            bias=ab_sb,
        )

        # Per-head logit upper bound, negated, +8: Exp bias (softmax is
        # shift-invariant); keeps exp outputs in (0, e^8] — fp16-safe.
        mx1 = spool.tile([H, 1], F32, tag="mx1")
        nc.vector.reduce_max(
            out=mx1[:], in_=s1_sb[:], axis=mybir.AxisListType.X, negate=True
        )
        mx2 = spool.tile([H, 1], F32, tag="mx2")
        nc.vector.reduce_max(
            out=mx2[:], in_=s2b_sb[:], axis=mybir.AxisListType.X, negate=True
        )
        nbound = spool.tile([H, 1], F32, tag="nbound")
        nc.vector.tensor_add(nbound[:], mx1[:], mx2[:])
        nc.vector.tensor_scalar_add(nbound[:], nbound[:], 8.0)
        # broadcast -bound to [P, H] via a DRAM roundtrip (stride-0 DMA)
        nc.sync.dma_start(out=nbd[b], in_=nbound[:])
        nbcols = spool.tile([P, H], F32, tag="nbcols")
        nbrow = nbd[b]
        nc.gpsimd.dma_start(
            out=nbcols[:],
            in_=bass.AP(
                tensor=nbd.tensor, offset=nbrow.offset,
                ap=[[0, P], nbrow.ap[-1]],
            ),
        )

        # s2b columns: [P, H] per j-chunk (PE transpose of [8, 128] slices)
        s2bT = []
        p_t4 = pset.tile([P, NCH, H], F32, tag="setup")
        for c in range(NCH):
            nc.tensor.transpose(p_t4[:, c, 0:H], s2b_sb[:, bass.ts(c, P)], ident8)
        for c in range(NCH):
            st = s2tpool.tile([P, H], F32, tag="s2T")
            nc.scalar.activation(st[:], p_t4[:, c, 0:H], mybir.ActivationFunctionType.Copy)
            s2bT.append(st)

        # h tiles + 8.0 column, fp16: aggregation rhs [h | 8]; the 8.0
        # rowsum column folds the mean-over-heads into 1/(8*rowsum)
        h16 = []
        p_h4 = pset.tile([P, NCH, FOUT], F32, tag="setup")
        for c in range(NCH):
            nc.tensor.matmul(
                p_h4[:, c, :], x_sb[:, bass.ts(c, P)], W_sb, start=True, stop=True
            )
        for c in range(NCH):
            ht = hpool.tile([P, FOUT + 1], F16, tag="h16")
            nc.scalar.activation(
                ht[:, 0:FOUT], p_h4[:, c, :], mybir.ActivationFunctionType.Copy
            )
            nc.vector.memset(ht[:, FOUT : FOUT + 1], float(H))
            h16.append(ht)

        acc = pacc.tile([P, NCH * FOUT], F32, tag="acc")
        G.append(dict(s2bT=s2bT, h16=h16, acc=acc, nbcols=nbcols,
                      s1ball=s1ball))

    # masks are needed ~10us later than consts/x: issue their DMAs after
    # everything else so the shared DMA engines finish the critical input
    # tensors first (packets round-robin across engines otherwise)
    for b in range(NB):
        m_sb = []
        for c in range(NCH):
            mt = mpool.tile([P, D], BF16, tag="mask")
            nc.sync.dma_start(out=mt, in_=maskT[b, c])
            m_sb.append(mt)
        G[b]["m_sb"] = m_sb

    # --- main per-head loop, graphs interleaved for deeper ILP ------------
    for hd in range(H):
        for b in range(NB):
            m_sb, s2bT = G[b]["m_sb"], G[b]["s2bT"]
            h16, acc, nbcols = G[b]["h16"], G[b]["acc"], G[b]["nbcols"]
            s1b = G[b]["s1ball"][hd][:]

            # u = leaky_relu(maskT + s2b[j] + S1B): one fused DVE pass
            u = upool.tile([P, NCH * D], F32, tag="u")
            for c in range(NCH):
                nc.vector._custom_dve(
                    fused_op,
                    out=u[:, bass.ts(c, D)],
                    in0=m_sb[c][:],
                    in1=s1b,
                    s0=s2bT[c][:, hd : hd + 1],
                    imm2=0.01,
                )

            # E = exp(u - bound + 8) in fp16
            E = epool.tile([P, NCH * D], F16, tag="E")
            nc.scalar.activation(
                E[:], u[:], mybir.ActivationFunctionType.Exp,
                bias=nbcols[:, hd : hd + 1],
            )

            # agg: U[t] = sum_c E^T[c, t]^T @ [h16[c] | 8]: FD=129, rowsums
            # ride col 128; two i-tiles share each PSUM bank
            p_Ub = []
            for tb in range(NCH // 2):
                p_Ubt = pout.tile([P, 2, FOUT + 1], F32, tag="pu")
                p_Ub.append(p_Ubt)
            p_Us = [p_Ub[t // 2][:, t % 2, :] for t in range(NCH)]
            for t in range(NCH):
                for c in range(NCH):
                    sl = E[:, c * D + t * P : c * D + (t + 1) * P]
                    nc.tensor.matmul(
                        p_Us[t], sl, h16[c][:],
                        start=(c == 0), stop=(c == NCH - 1),
                    )

            rcs = []
            for tb in range(NCH // 2):
                rc = rpool.tile([P, 2], F32, tag="rc")
                nc.vector.reciprocal(rc[:], p_Ub[tb][:, :, FOUT])
                rcs.append(rc)
            rcols = [rcs[t // 2][:, t % 2 : t % 2 + 1] for t in range(NCH)]

            tmp = tpool.tile([P, NCH * FOUT], F16, tag="tmp")
            for t in range(N_ACT_MERGE):
                nc.scalar.activation(
                    tmp[:, bass.ts(t, FOUT)], p_Us[t][:, 0:FOUT],
                    mybir.ActivationFunctionType.Copy,
                    scale=rcols[t],
                )
            for t in range(N_ACT_MERGE, NCH):
                nc.vector.tensor_scalar(
                    out=tmp[:, bass.ts(t, FOUT)], in0=p_Us[t][:, 0:FOUT],
                    scalar1=rcols[t], scalar2=None, op0=mult,
                )
            # head-accumulate on PE: acc += I @ tmp (PSUM accumulation)
            nc.tensor.matmul(
                acc[:], I128[:], tmp[:], start=(hd == 0), stop=(hd == H - 1),
            )

    for b in range(NB):
        accs = apool.tile([P, NCH * FOUT], F16, tag="accs")
        nc.scalar.activation(
            accs[:], G[b]["acc"][:], mybir.ActivationFunctionType.Copy
        )
        nc.sync.dma_start(out=out[b], in_=accs[:])


def _prep_core_inputs(input, adj, W, a_w, a_b, core):
    gs = slice(core * NB, (core + 1) * NB)
    x_c = input[gs]                                   # [NB, D, FIN]
    adj_c = adj[gs]                                   # [NB, D, D] int32
    xT = np.ascontiguousarray(x_c.transpose(0, 2, 1)).astype(np.float32)
    adjT = adj_c.transpose(0, 2, 1)                   # [NB, j, i]
    import ml_dtypes

    maskT = np.where(adjT > 0, np.float32(0.0), np.float32(NEG))
    maskT = np.ascontiguousarray(
        maskT.reshape(NB, NCH, P, D).astype(ml_dtypes.bfloat16)
    )
    return {
        "xT": xT,
        "maskT": maskT,
        "consts": _pack_consts(W, a_w, a_b),
    }


def _pack_consts(W, a_w, a_b):
    c = np.zeros((P, CONST_COLS), dtype=np.float32)
    c[:, 0:FOUT] = W
    c[:, FOUT : 2 * FOUT] = W.T
    c[:, 2 * FOUT : 2 * FOUT + H] = a_w[:, :FOUT].T
    c[:, 2 * FOUT + H : 2 * FOUT + 2 * H] = a_w[:, FOUT:].T
    c[0:H, 2 * FOUT + 2 * H] = a_b
    s0 = 2 * FOUT + 2 * H + 1
    c[0:H, s0 : s0 + H] = np.eye(H)
    c[:, s0 + H : s0 + H + P] = np.eye(P)
    return c


def get_nc():
    if "nc" not in _NC_CACHE:
        _NC_CACHE["nc"] = _build_bass()
    return _NC_CACHE["nc"]


def run_on_device(in_maps, **kwargs):
    return run_bass_kernel_spmd(get_nc(), in_maps, list(range(NCORES)), **kwargs)


def kernel(input, adj, W, a_w, a_b):
    input = np.asarray(input, dtype=np.float32)
    adj = np.asarray(adj)
    W = np.asarray(W, dtype=np.float32)
    a_w = np.asarray(a_w, dtype=np.float32)
    a_b = np.asarray(a_b, dtype=np.float32)

    in_maps = [
        _prep_core_inputs(input, adj, W, a_w, a_b, c) for c in range(NCORES)
    ]
    res = run_on_device(in_maps)
    outs = []
    for c in range(NCORES):
        a = np.asarray(res.results[c]["out"], dtype=np.float32)
        a = a.reshape(NB, P, NCH, FOUT).transpose(0, 2, 1, 3).reshape(NB, D, FOUT)
        outs.append(a)
    return np.concatenate(outs, axis=0)


if __name__ == "__main__":
    nc = get_nc()
    print("built ok")
